# revision 3
# baseline (speedup 1.0000x reference)
"""HEPT attention-score kernel for Trainium2 (8 NeuronCores, SPMD).

Computes out[b,h,i,j] = exp(min(q_i.k_j - 0.5||q_i||^2 - 0.5||k_j||^2, 0))
for B=2, H=8, S=2048, D=64 (fp32).

Sharding: the 16 (b,h) heads are split 2-per-core across 8 cores; each core
computes its two full 2048x2048 score tiles independently (no collectives).

Device math (per head, per 128-row query tile):
  PSUM = Qh.Kh + 1*nksq_h + 1*nksq_l      one fp16 matmul, K=66
  out  = Exp(PSUM + bias(-0.5||q||^2 + 10))  ScalarE activation -> fp16
Host divides by e^10 while widening fp16 -> fp32.

Precision: logits are <= 0 mathematically (max observed ~ -10.7), so
exp(logit+10) <= e^10 never overflows fp16. Error budget is 2e-2 of the
output absmax; fp16 rounding of q,k gives logit error sigma ~3e-3 and the
fp16 output quantization ~5e-4 relative -- measured 3.4e-3 global rel err.
The k-norm rides in the matmul as two extra contraction rows (hi+lo fp16
against all-ones lhs rows); the q-norm is the activation's exact fp32
per-partition bias. min(.,0) is dead code for these inputs.
"""

import numpy as np

B, H, S, D = 2, 8, 2048, 64
N_CORES = 8
HEADS_PER_CORE = (B * H) // N_CORES  # 2
P = 128              # partitions / rows per query tile
NT = S // P          # 16 query tiles per head
NCHUNK = 512         # matmul moving free dim (one PSUM bank of fp32)
NNC = S // NCHUNK    # 4 key chunks
KC = D + 2           # contraction rows: 64 q/k + nksq hi + nksq lo
EXP_C = 10.0         # logit offset keeping exp() in fp16 normal range


def _build_program():
    import concourse.bass as bass
    import concourse.bacc as bacc
    import concourse.mybir as mybir
    import concourse.tile as tile

    f16 = mybir.dt.float16
    f32 = mybir.dt.float32

    # Bacc (not raw Bass): its compile() pass splits multi-semaphore waits
    # into standalone event-sem instructions; walrus codegen rejects
    # instructions carrying more than the ISA's sync-wait slots.
    nc = bacc.Bacc("TRN2", target_bir_lowering=False, debug=False,
                   enable_asserts=False, num_devices=N_CORES)
    qt = nc.declare_dram_parameter(
        "qt", [HEADS_PER_CORE, KC, S], f16, isOutput=False)
    kt = nc.declare_dram_parameter(
        "kt", [HEADS_PER_CORE, KC, S], f16, isOutput=False)
    nqb = nc.declare_dram_parameter(
        "nqb", [HEADS_PER_CORE, P, NT], f32, isOutput=False)
    out = nc.declare_dram_parameter(
        "out", [HEADS_PER_CORE, S, S], f16, isOutput=True)

    with tile.TileContext(nc) as tc:
        with (
            tc.tile_pool(name="weights", bufs=2) as wpool,
            tc.tile_pool(name="bias", bufs=2) as bpool,
            tc.tile_pool(name="warm", bufs=1) as warmpool,
            tc.tile_pool(name="psum", bufs=2, space="PSUM") as ppool,
            tc.tile_pool(name="outs", bufs=3) as opool,
        ):
            # Dummy Exp at program start: walrus attaches the one-time ACT
            # table load here (it costs an extra sync-wait slot, which the
            # first real Activation cannot spare).
            warm = warmpool.tile([P, NT], f32)
            nc.vector.memset(warm[:], 0.0)
            nc.scalar.activation(warm[:], warm[:],
                                 mybir.ActivationFunctionType.Exp)

            for h in range(HEADS_PER_CORE):
                # First q-tile and k-chunk ride separate small DMAs so the
                # first matmul starts ~3us earlier; the bulk follows.
                qsA = wpool.tile([KC, P], f16, tag="qsA")
                nc.sync.dma_start(qsA[:], qt[h][:, 0:P])
                ksA = wpool.tile([KC, NCHUNK], f16, tag="ksA")
                nc.sync.dma_start(ksA[:], kt[h][:, 0:NCHUNK])
                nq = bpool.tile([P, NT], f32, tag="nq")
                nc.sync.dma_start(nq[:], nqb[h])
                ksB = wpool.tile([KC, S - NCHUNK], f16, tag="ksB")
                nc.sync.dma_start(ksB[:], kt[h][:, NCHUNK:])
                qsB = wpool.tile([KC, S - P], f16, tag="qsB")
                nc.sync.dma_start(qsB[:], qt[h][:, P:])

                for t in range(NT):
                    lhsT = qsA[:] if t == 0 else qsB[:, bass.ts(t - 1, P)]
                    ps = ppool.tile([P, S], f32)
                    for n in range(NNC):
                        rhs = (ksA[:] if n == 0
                               else ksB[:, bass.ts(n - 1, NCHUNK)])
                        nc.tensor.matmul(
                            ps[:, bass.ts(n, NCHUNK)], lhsT, rhs,
                            start=True, stop=True)
                    ob = opool.tile([P, S], f16)
                    nc.scalar.activation(
                        ob[:], ps[:], mybir.ActivationFunctionType.Exp,
                        bias=nq[:, t:t + 1], scale=1.0)
                    # two contiguous 256KB halves -> better queue overlap
                    nc.sync.dma_start(
                        out[h, t * P:t * P + P // 2], ob[0:P // 2, :])
                    nc.sync.dma_start(
                        out[h, t * P + P // 2:(t + 1) * P], ob[P // 2:P, :])
    nc.compile()
    return nc


def _prep_core(q, k):
    """q, k: [HEADS_PER_CORE, S, D] fp32 -> device input dict."""
    qh = q.astype(np.float16)
    kh = k.astype(np.float16)
    nqs = -0.5 * np.einsum("hsd,hsd->hs", q, q)          # [Hc, S] f32
    nks = -0.5 * np.einsum("hsd,hsd->hs", k, k)
    nks_h = nks.astype(np.float16)
    nks_l = (nks - nks_h.astype(np.float32)).astype(np.float16)

    ones2 = np.ones((HEADS_PER_CORE, 2, S), np.float16)
    qt = np.concatenate([qh.transpose(0, 2, 1), ones2], axis=1)  # [Hc,66,S]
    kt = np.concatenate(
        [kh.transpose(0, 2, 1), nks_h[:, None, :], nks_l[:, None, :]], axis=1)
    nqb = np.ascontiguousarray(
        (nqs + EXP_C).astype(np.float32)
        .reshape(HEADS_PER_CORE, NT, P).transpose(0, 2, 1))      # [Hc,P,NT]
    return {
        "qt": np.ascontiguousarray(qt),
        "kt": np.ascontiguousarray(kt),
        "nqb": nqb,
    }


_CACHE = {}


def kernel(query, key):
    from concourse.bass_utils import run_bass_kernel_spmd

    query = np.asarray(query, dtype=np.float32)
    key = np.asarray(key, dtype=np.float32)
    qf = query.reshape(B * H, S, D)
    kf = key.reshape(B * H, S, D)

    in_maps = []
    for c in range(N_CORES):
        sl = slice(c * HEADS_PER_CORE, (c + 1) * HEADS_PER_CORE)
        in_maps.append(_prep_core(qf[sl], kf[sl]))

    if "nc" not in _CACHE:
        _CACHE["nc"] = _build_program()
    res = run_bass_kernel_spmd(_CACHE["nc"], in_maps, list(range(N_CORES)))

    unscale = np.float32(np.exp(-EXP_C))
    out = np.empty((B * H, S, S), np.float32)
    for c in range(N_CORES):
        np.multiply(res.results[c]["out"], unscale,
                    out=out[c * HEADS_PER_CORE:(c + 1) * HEADS_PER_CORE],
                    casting="unsafe")
    return out.reshape(B, H, S, S)
